# revision 27
# baseline (speedup 1.0000x reference)
"""Trainium2 Bass kernel for a spatial self-attention block (fp8 DoubleRow).

reference computation (B=4, H=W=64, C=512, N=H*W=4096):
    h = group_norm(x, gamma, beta, 32 groups)
    q,k,v = h@wq+bq, h@wk+bk, h@wv+bv
    scores = (q @ k^T) / sqrt(C); attn = softmax(scores, -1)
    out = (attn @ v) @ wo + bo + x

Sharding: 8 cores = (batch b in 0..3) x (query-half in 0..1). Each core
computes group-norm stats + K/V for its full batch element (duplicated
across the pair) and attention outputs for its own 2048 query rows. The
host permutes each core's batch rows so its own queries are rows 0:2048.

All heavy matmuls run in fp8(e4m3) with perf_mode=DoubleRow: operands are
3D APs [128, 2, free] and the PE contracts over (partition x pair), giving
2 MACs/cell/cycle (~1.8x fp16 matmul throughput at free-dim 512).

Precision scheme (validated vs the fp32 reference: rel err ~9e-3 against a
2e-2 budget):
  - x arrives pre-transposed and pre-pair-interleaved from the host in fp8.
  - group-norm stats come from fp8 x and fp8 squares via DoubleRow matmuls
    against an all-ones stationary; scale/shift s,t are fp32 on-device.
  - s is folded into fp8 copies of wq/wk/wv scaled by WS=32 (weight entries
    ~N(0, 1/C) are too small for e4m3 otherwise); the 1/WS is applied in
    the PSUM->SBUF copy.  t is folded into effective biases (t@w + b).
  - exp uses a fixed shift: ex = exp(s/sqrt(C) - SHIFT), stored fp8
    (max scaled score measured ~6.8 -> e^4.8 = 127 < 240 = e4m3 max).
    The shift cancels in softmax normalization.
  - attn@V is computed unnormalized; V bias enters as bv_eff (x) denom
    (rows of unnormalized softmax sum to denom); the result is scaled by
    AOS=1/64 into fp8 for the O-projection, and 1/(WS*AOS*denom) is
    applied per-query after the O-projection.
"""

import sys

import numpy as np
import ml_dtypes

if "/opt/trn_rl_repo" not in sys.path:
    sys.path.insert(0, "/opt/trn_rl_repo")

import concourse.mybir as mybir
import concourse.tile as tile
from concourse import bacc
from concourse.bass_utils import run_bass_kernel_spmd

F32 = mybir.dt.float32
F32R = mybir.dt.float32r
F16 = mybir.dt.float16
F8 = mybir.dt.float8e4
AF = mybir.ActivationFunctionType
DR = mybir.MatmulPerfMode.DoubleRow
MULT = mybir.AluOpType.mult
ADD = mybir.AluOpType.add

B, N, C = 4, 4096, 512
HALF = N // 2          # own query rows per core
G, GS = 32, 16         # groups, channels per group
P = 128                # partitions
CO = C // P            # channel subtiles (4)
N_CORES = 8
EPS = 1e-6
SM = 1.0 / float(np.sqrt(C))
WS = 32.0              # weight fp8 scale
SHIFT = 2.0            # exp shift (cancels in softmax)
AOS = 1.0 / 64.0       # attn-output fp8 scale
ICH = 512              # query chunk
NCH = HALF // ICH      # 4
JT = N // P            # 32 key tiles
RT = N // 256          # 16 row-pair tiles (stats)
F8NP = ml_dtypes.float8_e4m3


def _r(ap):
    return ap.bitcast(F32R)


def build_nc():
    nc = bacc.Bacc("TRN2", target_bir_lowering=False, num_devices=N_CORES)

    xT8_d = nc.dram_tensor("xT8", [C, N], F8, kind="ExternalInput")
    x8i_d = nc.dram_tensor("x8i", [RT * P, 2 * C], F8, kind="ExternalInput")
    wq16_d = nc.dram_tensor("wq16", [P, CO, C], F16, kind="ExternalInput")
    wk16_d = nc.dram_tensor("wk16", [P, CO, C], F16, kind="ExternalInput")
    wv16_d = nc.dram_tensor("wv16", [P, CO, C], F16, kind="ExternalInput")
    wo8_d = nc.dram_tensor("wo8", [P, CO, C], F8, kind="ExternalInput")
    rows_d = nc.dram_tensor("rows", [1, 5 * C], F32, kind="ExternalInput")
    cst_d = nc.dram_tensor("cst", [P, 3], F32R, kind="ExternalInput")
    xbo_d = nc.dram_tensor("xbo", [HALF, C], F16, kind="ExternalInput")
    out_d = nc.dram_tensor("out", [HALF, C], F16, kind="ExternalOutput")

    x8i_b = x8i_d[:].rearrange("(b t p) c -> b p t c", t=4, p=P)  # 4x[128,4,1024]
    xbo_t = xbo_d[:].rearrange("(t p) c -> t p c", p=P)   # 16 x [128, 512]
    out_t = out_d[:].rearrange("(t p) c -> t p c", p=P)   # 16 x [128, 512]

    with tile.TileContext(nc) as tc:
        with (
            tc.tile_pool(name="persist", bufs=1) as persist,
            tc.tile_pool(name="cpool", bufs=1) as cpool,
        ):
            xT8 = persist.tile([P, CO, N], F8, tag="xT8")
            kT8 = persist.tile([P, CO, N], F8, tag="kT8")
            qT8 = persist.tile([P, CO, HALF], F8, tag="qT8")
            v8 = persist.tile([P, JT, C], F8, tag="v8")

            cst = cpool.tile([P, 3], F32R, tag="cst")
            ones8 = cpool.tile([P, 2, P], F8, tag="ones8")
            wo8 = cpool.tile([P, CO, C], F8, tag="wo8")
            w8 = {n: cpool.tile([P, CO, C], F8, tag=f"w8{n}", name=f"w8{n}")
                  for n in ("wq", "wk", "wv")}
            bqe_pp = cpool.tile([P, CO], F32, tag="bqe")
            bke_pp = cpool.tile([P, CO], F32, tag="bke")
            bv_eff = cpool.tile([1, C], F32R, tag="bve")

            nc.sync.dma_start(cst[:], cst_d[:])
            nc.gpsimd.memset(ones8[:], 1.0)
            ones_col = cst[:, 0:1]            # F32R
            ones_11 = cst[0:1, 0:1]           # F32R
            ones_11f = cst[0:1, 0:1].bitcast(F32)
            shift_col = cst[:, 1:2].bitcast(F32)
            eps_col = cst[:, 2:3].bitcast(F32)

            # ---- phase 1: group-norm stats + weight folding ----
            with (
                tc.tile_pool(name="w16p", bufs=1) as w16p,
                tc.tile_pool(name="xstage", bufs=4) as xstage,
                tc.tile_pool(name="sqpool", bufs=4) as sqpool,
                tc.tile_pool(name="prows", bufs=1) as prows,
                tc.tile_pool(name="stats_ps", bufs=1, space="PSUM") as stats_ps,
                tc.tile_pool(name="pize_ps", bufs=1, space="PSUM") as pize_ps,
                tc.tile_pool(name="warm_ps", bufs=1, space="PSUM") as warm_ps,
            ):
                # keep the PE busy from t~0 so the HAM clock gate opens
                # (K=8/8) before the real matmuls arrive; result never read.
                def warm(n, tag):
                    w = warm_ps.tile([P, P], F32, tag="warm", name=tag)
                    for wi in range(n):
                        nc.tensor.matmul(w[:], ones8[:], ones8[:],
                                         perf_mode=DR,
                                         start=(wi == 0), stop=(wi == n - 1),
                                         skip_group_check=True)

                warm(24, "w0")
                w16 = {n: w16p.tile([P, CO, C], F16, tag=f"w16{n}",
                                    name=f"w16{n}")
                       for n in ("wq", "wk", "wv")}

                irows = prows.tile([1, 5 * C], F32, tag="irows")
                nc.sync.dma_start(irows[:], rows_d[:])
                gamma_row = irows[:, 0 * C:1 * C]
                beta_row = irows[:, 1 * C:2 * C]
                bq_row = irows[:, 2 * C:3 * C]
                bk_row = irows[:, 3 * C:4 * C]
                bv_row = irows[:, 4 * C:5 * C]
                wrows = prows.tile([1, 4 * C], F32, tag="wrows")
                sum_row = wrows[:, 0 * C:1 * C]
                sq_row = wrows[:, 1 * C:2 * C]
                s_row = wrows[:, 2 * C:3 * C]
                t_row = wrows[:, 3 * C:4 * C]
                grows = prows.tile([1, 3 * G], F32, tag="grows")
                g_mean = grows[:, 0:G]
                g_var = grows[:, G:2 * G]
                g_tmp = grows[:, 2 * G:3 * G]
                stpart = prows.tile([P, 2 * CO], F32, tag="stpart")
                s32_part = stpart[:, 0:CO]
                t_partf = stpart[:, CO:2 * CO]
                t16_part = prows.tile([P, CO], F16, tag="t16")

                # stats: column sums and sums-of-squares via DoubleRow
                s_ps = stats_ps.tile([P, C], F32, tag="S")
                q_ps = stats_ps.tile([P, C], F32, tag="Q")
                NB = 4  # row-pair tiles per DMA batch
                sq_eng = [nc.vector, nc.gpsimd, nc.scalar, nc.scalar,
                          nc.vector, nc.gpsimd, nc.scalar, nc.scalar,
                          nc.vector, nc.gpsimd, nc.scalar, nc.scalar,
                          nc.scalar, nc.scalar, nc.scalar, nc.scalar]
                sqs = []
                for b in range(RT // NB):
                    xt = xstage.tile([P, NB, 2, C], F8, tag="xt",
                                     name=f"xt{b}")
                    sq = sqpool.tile([P, NB, 2, C], F8, tag="sq",
                                     name=f"sq{b}")
                    sqs.append(sq)
                    if b % 2 == 0:
                        nc.sync.dma_start(
                            xt[:].rearrange("p t two c -> p t (two c)"),
                            x8i_b[b])
                    else:
                        nc.gpsimd.dma_start(
                            xt[:].rearrange("p t two c -> p t (two c)"),
                            x8i_b[b])
                    for t in range(NB):
                        g = b * NB + t
                        nc.tensor.matmul(s_ps[:], ones8[:], xt[:, t],
                                         perf_mode=DR,
                                         start=(g == 0), stop=(g == RT - 1))
                        e = sq_eng[g]
                        if e is nc.scalar:
                            e.activation(sq[:, t], xt[:, t], AF.Square)
                        else:
                            e.tensor_mul(sq[:, t], xt[:, t], xt[:, t])
                    warm(6, f"wb{b}")
                # squares complete on scalar/vector/gpsimd while the sum
                # matmuls stream; the sumsq matmuls then run back-to-back
                for g in range(RT):
                    nc.tensor.matmul(q_ps[:], ones8[:], sqs[g // NB][:, g % NB],
                                     perf_mode=DR,
                                     start=(g == 0), stop=(g == RT - 1))
                for o in range(CO):
                    nc.sync.dma_start(xT8[:, o, :],
                                      xT8_d[o * P:(o + 1) * P, :])
                for name, srcd in (("wk", wk16_d), ("wq", wq16_d),
                                   ("wv", wv16_d)):
                    nc.sync.dma_start(w16[name][:], srcd[:])
                nc.sync.dma_start(wo8[:], wo8_d[:])

                # warm filler: PE stays hot while the rows chain runs
                warm(24, "w1")

                # group stats -> per-channel scale/shift (rows, DVE)
                inv_cnt = 1.0 / (N * GS)
                nc.vector.reduce_sum(g_mean,
                                     s_ps[0:1, :].rearrange(
                                         "p (g e) -> p g e", e=GS),
                                     axis=mybir.AxisListType.X)
                nc.vector.tensor_scalar_mul(g_mean, g_mean, inv_cnt)
                nc.vector.reduce_sum(g_var,
                                     q_ps[0:1, :].rearrange(
                                         "p (g e) -> p g e", e=GS),
                                     axis=mybir.AxisListType.X)
                nc.vector.tensor_mul(g_tmp, g_mean, g_mean)
                nc.vector.scalar_tensor_tensor(g_var, g_var, inv_cnt, g_tmp,
                                               MULT, mybir.AluOpType.subtract)
                nc.scalar.activation(g_tmp, g_var, AF.Sqrt,
                                     bias=eps_col[0:1, :])
                nc.vector.reciprocal(g_tmp, g_tmp)  # rstd per group

                sv = s_row.rearrange("p (g e) -> p g e", e=GS)
                tv = t_row.rearrange("p (g e) -> p g e", e=GS)
                gv = gamma_row.rearrange("p (g e) -> p g e", e=GS)
                nc.vector.tensor_tensor(
                    sv, gv, g_tmp[:, :, None].to_broadcast((1, G, GS)), MULT)
                nc.vector.tensor_tensor(
                    tv, sv, g_mean[:, :, None].to_broadcast((1, G, GS)), MULT)
                nc.vector.tensor_sub(t_row, beta_row, t_row)
                nc.vector.tensor_scalar_mul(s_row, s_row, WS)  # 32*s

                # partition-ize s32, t  ([1,512] row -> [128,4])
                for vec_row, dst in ((s_row, s32_part), (t_row, t_partf)):
                    pp = pize_ps.tile([P, CO], F32, tag="pize", name="pp")
                    for o in range(CO):
                        nc.tensor.matmul(pp[:, o:o + 1],
                                         vec_row[0:1, o * P:(o + 1) * P],
                                         ones_11f,
                                         start=(o == 0), stop=(o == CO - 1))
                    nc.vector.tensor_copy(dst, pp[:])
                nc.vector.tensor_copy(t16_part[:], t_partf)

                # effective biases b' = t @ W + b
                beff_rows = prows.tile([1, 3 * C], F32, tag="beff")
                for i, (name, brow) in enumerate(
                        (("wq", bq_row), ("wk", bk_row), ("wv", bv_row))):
                    bps = stats_ps.tile([1, C], F32, tag="S", name=f"bps{i}")
                    for o in range(CO):
                        nc.tensor.matmul(bps[:], t16_part[:, o:o + 1],
                                         w16[name][:, o, :],
                                         start=(o == 0), stop=(o == CO - 1))
                    nc.vector.tensor_add(beff_rows[:, i * C:(i + 1) * C],
                                         bps[:], brow)
                for i, dst in ((0, bqe_pp), (1, bke_pp)):
                    vec_row = beff_rows[:, i * C:(i + 1) * C]
                    pp = pize_ps.tile([P, CO], F32, tag="pize", name="pp")
                    for o in range(CO):
                        nc.tensor.matmul(pp[:, o:o + 1],
                                         vec_row[0:1, o * P:(o + 1) * P],
                                         ones_11f,
                                         start=(o == 0), stop=(o == CO - 1))
                    nc.vector.tensor_copy(dst[:], pp[:])
                nc.vector.tensor_copy(bv_eff[:], beff_rows[:, 2 * C:3 * C])

                warm(14, "w2a")

                # fold 32*s into fp8 weights
                for i, name in enumerate(("wk", "wq", "wv")):
                    for ci in range(CO):
                        if (i * CO + ci) % 2 == 0:
                            nc.scalar.activation(w8[name][:, ci, :],
                                                 w16[name][:, ci, :], AF.Copy,
                                                 scale=s32_part[:, ci:ci + 1])
                        else:
                            nc.vector.tensor_scalar_mul(
                                w8[name][:, ci, :], w16[name][:, ci, :],
                                s32_part[:, ci:ci + 1])

            # ---- phase 2: Q/K/V projections (DoubleRow fp8) ----
            # K, Q window 0 and V tiles 0..19 are produced here; the
            # remaining V tiles and Q windows migrate into the attention
            # chunks (emitted as `extras`) where the copy engines have slack.
            cpc = [0]

            def copy_sb(dst, srcp, bias_col):
                # gpsimd cannot read PSUM: rotate scalar/vector 4:3
                e = 0 if cpc[0] % 7 < 4 else 1
                cpc[0] += 1
                if bias_col is None:
                    if e == 0:
                        nc.scalar.activation(dst, srcp, AF.Copy,
                                             scale=1.0 / WS)
                    else:
                        nc.vector.tensor_scalar_mul(dst, srcp, 1.0 / WS)
                else:
                    if e == 0:
                        nc.scalar.activation(dst, srcp, AF.Identity,
                                             bias=bias_col, scale=1.0 / WS)
                    else:
                        nc.vector.tensor_scalar(dst, srcp, 1.0 / WS,
                                                bias_col, MULT, ADD)

            def kq_group(pool, ptag, w8t, dstT, bias_pp, o, win, nm):
                ps = pool.tile([P, ICH], F32, tag=ptag,
                               name=f"{nm}{o}_{win}")
                for u in range(2):
                    nc.tensor.matmul(
                        ps[:],
                        w8t[:, 2 * u:2 * u + 2, o * P:(o + 1) * P],
                        xT8[:, 2 * u:2 * u + 2, win * ICH:(win + 1) * ICH],
                        perf_mode=DR, start=(u == 0), stop=(u == 1))
                copy_sb(dstT[:, o, win * ICH:(win + 1) * ICH], ps[:],
                        bias_pp[:, o:o + 1])

            def v_group(pool, ptag, t):
                ps = pool.tile([P, C], F32, tag=ptag, name=f"v{t}")
                for u in range(2):
                    nc.tensor.matmul(
                        ps[:],
                        xT8[:, 2 * u:2 * u + 2, t * P:(t + 1) * P],
                        w8["wv"][:, 2 * u:2 * u + 2, :],
                        perf_mode=DR, start=(u == 0), stop=(u == 1))
                copy_sb(v8[:, t, :], ps[:], None)

            with tc.tile_pool(name="proj_ps", bufs=6, space="PSUM") as proj_ps:
                warm2 = proj_ps.tile([P, P], F32, tag="proj", name="warm2")
                for wi in range(16):
                    nc.tensor.matmul(warm2[:], ones8[:], ones8[:],
                                     perf_mode=DR,
                                     start=(wi == 0), stop=(wi == 15),
                                     skip_group_check=True)
                for o in range(CO):
                    for win in range(N // ICH):
                        kq_group(proj_ps, "proj", w8["wk"], kT8, bke_pp, o, win, "k")
                for o in range(CO):
                    kq_group(proj_ps, "proj", w8["wq"], qT8, bqe_pp, o, 0, "q")
                for t in range(20):
                    v_group(proj_ps, "proj", t)

            # ---- phase 3: attention + O-projection + residual ----
            with (
                tc.tile_pool(name="av_ps", bufs=1, space="PSUM") as av_ps,
                tc.tile_pool(name="sps_ps", bufs=3, space="PSUM") as sps_ps,
                tc.tile_pool(name="op_ps", bufs=1, space="PSUM") as op_ps,
                tc.tile_pool(name="expp", bufs=3) as expp,
                tc.tile_pool(name="accp", bufs=2) as accp,
                tc.tile_pool(name="aoTp", bufs=2) as aoTp,
                tc.tile_pool(name="drow", bufs=2) as drow,
                tc.tile_pool(name="xres", bufs=6) as xres,
                tc.tile_pool(name="ostage", bufs=2) as ostage,
            ):
                LAG = 3  # AV pairs trail scores by 3 so tail MMs interleave

                def make_tail(ch, avs, acc_a, acc_b, last=False):
                    """Chunk-end work, split into pieces emitted between the
                    next chunk's score matmuls (PE queue is in-order, so the
                    tail's dependency waits must be covered by stream MMs)."""
                    st = {}

                    pool, ptag = (sps_ps, "sps") if last else (op_ps, "op")
                    if last:
                        xrs = []
                        for it in range(CO):
                            xr = xres.tile([P, C], F16, tag="xr",
                                           name=f"xrL{it}")
                            nc.sync.dma_start(xr[:], xbo_t[ch * CO + it])
                            xrs.append(xr)

                    def p0():
                        nc.vector.tensor_add(acc_a[:], acc_a[:], acc_b[:])
                        dps = pool.tile([1, ICH], F32, tag=ptag,
                                        name=f"dps{ch}")
                        nc.tensor.matmul(dps[:], ones_col.bitcast(F32),
                                         acc_a[:], start=True, stop=True)
                        d_row = drow.tile([1, ICH], F32R, tag="d_row",
                                          name=f"drow{ch}")
                        nc.vector.tensor_copy(d_row[:], dps[:])
                        st["d_row"] = d_row

                    def p1():
                        d_row = st["d_row"]
                        for cs in range(CO):
                            nc.tensor.matmul(
                                avs[cs][:],
                                bv_eff[0:1, cs * P:(cs + 1) * P], d_row[:],
                                start=False, stop=True)

                    def p2():
                        d_row = st["d_row"]
                        dp = pool.tile([P, CO], F32, tag=ptag, name=f"dp{ch}")
                        for o in range(CO):
                            nc.tensor.matmul(dp[:, o:o + 1],
                                             d_row[0:1, o * P:(o + 1) * P]
                                             .bitcast(F32),
                                             ones_11f,
                                             start=(o == 0),
                                             stop=(o == CO - 1))
                        d_inv = drow.tile([P, CO], F32, tag="d_inv",
                                          name=f"dinv{ch}")
                        nc.vector.tensor_scalar_mul(d_inv[:], dp[:], WS * AOS)
                        nc.vector.reciprocal(d_inv[:], d_inv[:])
                        aoT = aoTp.tile([P, CO, ICH], F8, tag="aoT",
                                        name=f"aoT{ch}")
                        for cs in range(CO):
                            if cs % 2 == 0:
                                nc.vector.tensor_scalar_mul(aoT[:, cs, :],
                                                            avs[cs][:], AOS)
                            else:
                                nc.scalar.activation(aoT[:, cs, :],
                                                     avs[cs][:], AF.Copy,
                                                     scale=AOS)
                        st["d_inv"] = d_inv
                        st["aoT"] = aoT

                    def mk_it(it):
                        def p():
                            aoT, d_inv = st["aoT"], st["d_inv"]
                            ops = pool.tile([P, C], F32, tag=ptag,
                                            name=f"o{ch}_{it}")
                            for u in range(2):
                                nc.tensor.matmul(
                                    ops[:],
                                    aoT[:, 2 * u:2 * u + 2,
                                        it * P:(it + 1) * P],
                                    wo8[:, 2 * u:2 * u + 2, :],
                                    perf_mode=DR, start=(u == 0),
                                    stop=(u == 1))
                            if last:
                                xr = xrs[it]
                            else:
                                xr = xres.tile([P, C], F16, tag="xr",
                                               name=f"xr{ch}_{it}")
                                nc.sync.dma_start(xr[:], xbo_t[ch * CO + it])
                            ot = ostage.tile([P, C], F16, tag="ot",
                                             name=f"ot{ch}_{it}")
                            nc.vector.scalar_tensor_tensor(
                                ot[:], ops[:], d_inv[:, it:it + 1], xr[:],
                                MULT, ADD)
                            nc.sync.dma_start(out_t[ch * CO + it], ot[:])
                        return p

                    return [p0, p1, p2, mk_it(0), mk_it(1), mk_it(2),
                            mk_it(3)]

                extras_by_ch = {
                    0: ([("v", t) for t in range(20, JT)]
                        + [("q", o, 1) for o in range(CO)]),
                    1: [("q", o, 2) for o in range(CO)],
                    2: [("q", o, 3) for o in range(CO)],
                    3: [],
                }

                tail = []
                for ch in range(NCH):
                    i0 = ch * ICH
                    extras = list(extras_by_ch[ch])
                    avs = [av_ps.tile([P, ICH], F32, tag=f"av{i}",
                                      name=f"av{ch}_{i}")
                           for i in range(CO)]
                    acc_a = accp.tile([P, ICH], F32, tag="acc_a",
                                      name=f"acca{ch}")
                    acc_b = accp.tile([P, ICH], F32, tag="acc_b",
                                      name=f"accb{ch}")

                    def scores(j, ex, jj, i0=i0, acc_a=acc_a, acc_b=acc_b,
                               ch=ch):
                        sps = sps_ps.tile([P, ICH], F32, tag="sps",
                                          name=f"sps{ch}_{j}")
                        for u in range(2):
                            nc.tensor.matmul(
                                sps[:],
                                kT8[:, 2 * u:2 * u + 2, j * P:(j + 1) * P],
                                qT8[:, 2 * u:2 * u + 2, i0:i0 + ICH],
                                perf_mode=DR, start=(u == 0), stop=(u == 1))
                        nc.scalar.activation(ex[:, jj, :], sps[:], AF.Exp,
                                             bias=shift_col, scale=SM)
                        if jj == 0:
                            if j == 0:
                                nc.vector.tensor_copy(acc_a[:], ex[:, 0, :])
                            else:
                                nc.vector.tensor_add(acc_a[:], acc_a[:],
                                                     ex[:, 0, :])
                        else:
                            if j == 1:
                                nc.gpsimd.tensor_copy(acc_b[:], ex[:, 1, :])
                            else:
                                nc.gpsimd.tensor_add(acc_b[:], acc_b[:],
                                                     ex[:, 1, :])

                    def av_mms(t, ex, avs=avs):
                        for cs in range(CO):
                            nc.tensor.matmul(
                                avs[cs][:],
                                v8[:, 2 * t:2 * t + 2, cs * P:(cs + 1) * P],
                                ex[:],
                                perf_mode=DR, start=(t == 0), stop=False)

                    lag = 1 if ch == NCH - 1 else LAG
                    exs = {}
                    for t in range(JT // 2):
                        ex = expp.tile([P, 2, ICH], F8, tag="ex",
                                       name=f"ex{ch}_{t}")
                        exs[t] = ex
                        scores(2 * t, ex, 0)
                        scores(2 * t + 1, ex, 1)
                        if 1 <= t <= len(tail):
                            tail[t - 1]()
                        if extras:
                            e = extras.pop(0)
                            if e[0] == "v":
                                v_group(sps_ps, "sps", e[1])
                            else:
                                kq_group(sps_ps, "sps", w8["wq"], qT8,
                                         bqe_pp, e[1], e[2], "qx")
                        if t >= lag:
                            av_mms(t - lag, exs.pop(t - lag))
                    for t in range(JT // 2 - lag, JT // 2):
                        av_mms(t, exs.pop(t))
                    tail = make_tail(ch, avs, acc_a, acc_b,
                                     last=(ch == NCH - 1))
                def warm_tail(n, tag):
                    w = sps_ps.tile([P, P], F32, tag="sps", name=tag)
                    for wi in range(n):
                        nc.tensor.matmul(w[:], ones8[:], ones8[:],
                                         perf_mode=DR,
                                         start=(wi == 0), stop=(wi == n - 1),
                                         skip_group_check=True)

                for i, piece in enumerate(tail):
                    piece()
                    if i in (0, 2):
                        warm_tail(8, f"wt{i}")

    nc.compile()
    return nc


_NC = None


def _get_nc():
    global _NC
    if _NC is None:
        _NC = build_nc()
    return _NC


def make_in_maps(x, gn_gamma, gn_beta, wq, bq, wk, bk, wv, bv, wo, bo):
    x4 = np.asarray(x, np.float32).reshape(B, N, C)

    def wlay(w):
        return np.asarray(w, np.float32).reshape(CO, P, C).transpose(1, 0, 2)

    rows = np.zeros((1, 5 * C), np.float32)
    for i, v in enumerate((gn_gamma, gn_beta, bq, bk, bv)):
        rows[0, i * C:(i + 1) * C] = np.asarray(v, np.float32)
    cst = np.zeros((P, 3), np.float32)
    cst[:, 0] = 1.0
    cst[:, 1] = -SHIFT
    cst[:, 2] = EPS
    # col 2: group-norm eps (bias operand of the Sqrt activation)
    common = dict(
        wq16=np.ascontiguousarray(wlay(wq).astype(np.float16)),
        wk16=np.ascontiguousarray(wlay(wk).astype(np.float16)),
        wv16=np.ascontiguousarray(wlay(wv).astype(np.float16)),
        wo8=np.ascontiguousarray((WS * wlay(wo)).astype(F8NP)),
        rows=rows, cst=cst,
    )
    bo_f = np.asarray(bo, np.float32)
    in_maps = []
    for c in range(N_CORES):
        b, h = c // 2, c % 2
        own = x4[b, h * HALF:(h + 1) * HALF]
        other = x4[b, (1 - h) * HALF:(2 - h) * HALF]
        xp = np.concatenate([own, other], axis=0)        # [N, C]
        xT8 = np.ascontiguousarray(xp.T.astype(F8NP))    # [C, N]
        xi = xp.reshape(RT, 2, P, C).transpose(0, 2, 1, 3) \
               .reshape(RT * P, 2 * C)
        x8i = np.ascontiguousarray(xi.astype(F8NP))
        xbo = np.ascontiguousarray((own + bo_f).astype(np.float16))
        in_maps.append(dict(xT8=xT8, x8i=x8i, xbo=xbo, **common))
    return in_maps


def assemble(results):
    out = np.empty((B, N, C), np.float32)
    for c in range(N_CORES):
        b, h = c // 2, c % 2
        out[b, h * HALF:(h + 1) * HALF] = results[c]["out"].astype(np.float32)
    return out.reshape(B, 64, 64, C)


def kernel(**inputs):
    nc = _get_nc()
    in_maps = make_in_maps(**inputs)
    res = run_bass_kernel_spmd(nc, in_maps, list(range(N_CORES)))
    return assemble(res.results)


# revision 29
# speedup vs baseline: 1.1884x; 1.1884x over previous
"""Trainium2 Bass kernel for a spatial self-attention block (fp8 DoubleRow).

reference computation (B=4, H=W=64, C=512, N=H*W=4096):
    h = group_norm(x, gamma, beta, 32 groups)
    q,k,v = h@wq+bq, h@wk+bk, h@wv+bv
    scores = (q @ k^T) / sqrt(C); attn = softmax(scores, -1)
    out = (attn @ v) @ wo + bo + x

Sharding: 8 cores = (batch b in 0..3) x (query-half in 0..1). Each core
computes group-norm stats + K/V for its full batch element (duplicated
across the pair) and attention outputs for its own 2048 query rows. The
host permutes each core's batch rows so its own queries are rows 0:2048.

All heavy matmuls run in fp8(e4m3) with perf_mode=DoubleRow: operands are
3D APs [128, 2, free] and the PE contracts over (partition x pair), giving
2 MACs/cell/cycle (~1.8x fp16 matmul throughput at free-dim 512).

Precision scheme (validated vs the fp32 reference: rel err ~9e-3 against a
2e-2 budget):
  - x arrives pre-transposed and pre-pair-interleaved from the host in fp8.
  - group-norm stats come from fp8 x and fp8 squares via DoubleRow matmuls
    against an all-ones stationary; scale/shift s,t are fp32 on-device.
  - s is folded into fp8 copies of wq/wk/wv scaled by WS=32 (weight entries
    ~N(0, 1/C) are too small for e4m3 otherwise); the 1/WS is applied in
    the PSUM->SBUF copy.  t is folded into effective biases (t@w + b).
  - exp uses a fixed shift: ex = exp(s/sqrt(C) - SHIFT), stored fp8
    (max scaled score measured ~6.8 -> e^4.8 = 127 < 240 = e4m3 max).
    The shift cancels in softmax normalization.
  - attn@V is computed unnormalized; V bias enters as bv_eff (x) denom
    (rows of unnormalized softmax sum to denom); the result is scaled by
    AOS=1/64 into fp8 for the O-projection, and 1/(WS*AOS*denom) is
    applied per-query after the O-projection.
"""

import sys

import numpy as np
import ml_dtypes

if "/opt/trn_rl_repo" not in sys.path:
    sys.path.insert(0, "/opt/trn_rl_repo")

import concourse.mybir as mybir
import concourse.tile as tile
from concourse import bacc
from concourse.bass_utils import run_bass_kernel_spmd

F32 = mybir.dt.float32
F32R = mybir.dt.float32r
F16 = mybir.dt.float16
F8 = mybir.dt.float8e4
AF = mybir.ActivationFunctionType
DR = mybir.MatmulPerfMode.DoubleRow
MULT = mybir.AluOpType.mult
ADD = mybir.AluOpType.add

B, N, C = 4, 4096, 512
HALF = N // 2          # own query rows per core
G, GS = 32, 16         # groups, channels per group
P = 128                # partitions
CO = C // P            # channel subtiles (4)
N_CORES = 8
EPS = 1e-6
SM = 1.0 / float(np.sqrt(C))
WS = 32.0              # weight fp8 scale
SHIFT = 2.0            # exp shift (cancels in softmax)
AOS = 1.0 / 64.0       # attn-output fp8 scale
ICH = 512              # query chunk
NCH = HALF // ICH      # 4
JT = N // P            # 32 key tiles
RT = N // 256          # 16 row-pair tiles (stats)
F8NP = ml_dtypes.float8_e4m3


def _r(ap):
    return ap.bitcast(F32R)


def build_nc():
    nc = bacc.Bacc("TRN2", target_bir_lowering=False, num_devices=N_CORES)

    xT8_d = nc.dram_tensor("xT8", [C, N], F8, kind="ExternalInput")
    x8i_d = nc.dram_tensor("x8i", [RT * P, 2 * C], F8, kind="ExternalInput")
    wq16_d = nc.dram_tensor("wq16", [P, CO, C], F16, kind="ExternalInput")
    wk16_d = nc.dram_tensor("wk16", [P, CO, C], F16, kind="ExternalInput")
    wv16_d = nc.dram_tensor("wv16", [P, CO, C], F16, kind="ExternalInput")
    wo8_d = nc.dram_tensor("wo8", [P, CO, C], F8, kind="ExternalInput")
    rows_d = nc.dram_tensor("rows", [1, 5 * C], F32, kind="ExternalInput")
    cst_d = nc.dram_tensor("cst", [P, 3], F32R, kind="ExternalInput")
    xbo_d = nc.dram_tensor("xbo", [HALF, C], F16, kind="ExternalInput")
    out_d = nc.dram_tensor("out", [HALF, C], F16, kind="ExternalOutput")

    x8i_b = x8i_d[:].rearrange("(b t p) c -> b p t c", t=4, p=P)  # 4x[128,4,1024]
    xbo_t = xbo_d[:].rearrange("(t p) c -> t p c", p=P)   # 16 x [128, 512]
    out_t = out_d[:].rearrange("(t p) c -> t p c", p=P)   # 16 x [128, 512]

    with tile.TileContext(nc) as tc:
        with (
            tc.tile_pool(name="persist", bufs=1) as persist,
            tc.tile_pool(name="cpool", bufs=1) as cpool,
        ):
            xT8 = persist.tile([P, CO, N], F8, tag="xT8")
            kT8 = persist.tile([P, CO, N], F8, tag="kT8")
            qT8 = persist.tile([P, CO, HALF], F8, tag="qT8")
            v8 = persist.tile([P, JT, C], F8, tag="v8")

            cst = cpool.tile([P, 3], F32R, tag="cst")
            ones8 = cpool.tile([P, 2, P], F8, tag="ones8")
            wo8 = cpool.tile([P, CO, C], F8, tag="wo8")
            w8 = {n: cpool.tile([P, CO, C], F8, tag=f"w8{n}", name=f"w8{n}")
                  for n in ("wq", "wk", "wv")}
            bqe_pp = cpool.tile([P, CO], F32, tag="bqe")
            bke_pp = cpool.tile([P, CO], F32, tag="bke")
            bv_eff = cpool.tile([1, C], F32R, tag="bve")

            nc.sync.dma_start(cst[:], cst_d[:])
            nc.gpsimd.memset(ones8[:], 1.0)
            ones_col = cst[:, 0:1]            # F32R
            ones_11 = cst[0:1, 0:1]           # F32R
            ones_11f = cst[0:1, 0:1].bitcast(F32)
            shift_col = cst[:, 1:2].bitcast(F32)
            eps_col = cst[:, 2:3].bitcast(F32)

            # ---- phase 1: group-norm stats + weight folding ----
            with (
                tc.tile_pool(name="w16p", bufs=1) as w16p,
                tc.tile_pool(name="xstage", bufs=4) as xstage,
                tc.tile_pool(name="sqpool", bufs=4) as sqpool,
                tc.tile_pool(name="prows", bufs=1) as prows,
                tc.tile_pool(name="stats_ps", bufs=1, space="PSUM") as stats_ps,
                tc.tile_pool(name="pize_ps", bufs=1, space="PSUM") as pize_ps,
                tc.tile_pool(name="warm_ps", bufs=1, space="PSUM") as warm_ps,
            ):
                # keep the PE busy from t~0 so the HAM clock gate opens
                # (K=8/8) before the real matmuls arrive; result never read.
                def warm(n, tag):
                    w = warm_ps.tile([P, P], F32, tag="warm", name=tag)
                    for wi in range(n):
                        nc.tensor.matmul(w[:], ones8[:], ones8[:],
                                         perf_mode=DR,
                                         start=(wi == 0), stop=(wi == n - 1),
                                         skip_group_check=True)

                warm(24, "w0")
                w16 = {n: w16p.tile([P, CO, C], F16, tag=f"w16{n}",
                                    name=f"w16{n}")
                       for n in ("wq", "wk", "wv")}

                irows = prows.tile([1, 5 * C], F32, tag="irows")
                nc.sync.dma_start(irows[:], rows_d[:])
                gamma_row = irows[:, 0 * C:1 * C]
                beta_row = irows[:, 1 * C:2 * C]
                bq_row = irows[:, 2 * C:3 * C]
                bk_row = irows[:, 3 * C:4 * C]
                bv_row = irows[:, 4 * C:5 * C]
                wrows = prows.tile([1, 4 * C], F32, tag="wrows")
                sum_row = wrows[:, 0 * C:1 * C]
                sq_row = wrows[:, 1 * C:2 * C]
                s_row = wrows[:, 2 * C:3 * C]
                t_row = wrows[:, 3 * C:4 * C]
                grows = prows.tile([1, 3 * G], F32, tag="grows")
                g_mean = grows[:, 0:G]
                g_var = grows[:, G:2 * G]
                g_tmp = grows[:, 2 * G:3 * G]
                stpart = prows.tile([P, 2 * CO], F32, tag="stpart")
                s32_part = stpart[:, 0:CO]
                t_partf = stpart[:, CO:2 * CO]
                t16_part = prows.tile([P, CO], F16, tag="t16")

                # stats: column sums and sums-of-squares via DoubleRow
                s_ps = stats_ps.tile([P, C], F32, tag="S")
                q_ps = stats_ps.tile([P, C], F32, tag="Q")
                NB = 4  # row-pair tiles per DMA batch
                sq_eng = [nc.vector, nc.gpsimd, nc.scalar, nc.scalar,
                          nc.vector, nc.gpsimd, nc.scalar, nc.scalar,
                          nc.vector, nc.gpsimd, nc.scalar, nc.scalar,
                          nc.scalar, nc.scalar, nc.scalar, nc.scalar]
                sqs = []
                for b in range(RT // NB):
                    xt = xstage.tile([P, NB, 2, C], F8, tag="xt",
                                     name=f"xt{b}")
                    sq = sqpool.tile([P, NB, 2, C], F8, tag="sq",
                                     name=f"sq{b}")
                    sqs.append(sq)
                    if b % 2 == 0:
                        nc.sync.dma_start(
                            xt[:].rearrange("p t two c -> p t (two c)"),
                            x8i_b[b])
                    else:
                        nc.gpsimd.dma_start(
                            xt[:].rearrange("p t two c -> p t (two c)"),
                            x8i_b[b])
                    for t in range(NB):
                        g = b * NB + t
                        nc.tensor.matmul(s_ps[:], ones8[:], xt[:, t],
                                         perf_mode=DR,
                                         start=(g == 0), stop=(g == RT - 1))
                        e = sq_eng[g]
                        if e is nc.scalar:
                            e.activation(sq[:, t], xt[:, t], AF.Square)
                        else:
                            e.tensor_mul(sq[:, t], xt[:, t], xt[:, t])
                    warm(6, f"wb{b}")
                # squares complete on scalar/vector/gpsimd while the sum
                # matmuls stream; the sumsq matmuls then run back-to-back
                for g in range(RT):
                    nc.tensor.matmul(q_ps[:], ones8[:], sqs[g // NB][:, g % NB],
                                     perf_mode=DR,
                                     start=(g == 0), stop=(g == RT - 1))
                for o in range(CO):
                    nc.sync.dma_start(xT8[:, o, :],
                                      xT8_d[o * P:(o + 1) * P, :])
                for name, srcd in (("wk", wk16_d), ("wq", wq16_d),
                                   ("wv", wv16_d)):
                    nc.sync.dma_start(w16[name][:], srcd[:])
                nc.sync.dma_start(wo8[:], wo8_d[:])

                # warm filler: PE stays hot while the rows chain runs
                warm(24, "w1")

                # group stats -> per-channel scale/shift (rows, DVE)
                inv_cnt = 1.0 / (N * GS)
                nc.vector.reduce_sum(g_mean,
                                     s_ps[0:1, :].rearrange(
                                         "p (g e) -> p g e", e=GS),
                                     axis=mybir.AxisListType.X)
                nc.vector.tensor_scalar_mul(g_mean, g_mean, inv_cnt)
                nc.vector.reduce_sum(g_var,
                                     q_ps[0:1, :].rearrange(
                                         "p (g e) -> p g e", e=GS),
                                     axis=mybir.AxisListType.X)
                nc.vector.tensor_mul(g_tmp, g_mean, g_mean)
                nc.vector.scalar_tensor_tensor(g_var, g_var, inv_cnt, g_tmp,
                                               MULT, mybir.AluOpType.subtract)
                nc.scalar.activation(g_tmp, g_var, AF.Sqrt,
                                     bias=eps_col[0:1, :])
                nc.vector.reciprocal(g_tmp, g_tmp)  # rstd per group

                sv = s_row.rearrange("p (g e) -> p g e", e=GS)
                tv = t_row.rearrange("p (g e) -> p g e", e=GS)
                gv = gamma_row.rearrange("p (g e) -> p g e", e=GS)
                nc.vector.tensor_tensor(
                    sv, gv, g_tmp[:, :, None].to_broadcast((1, G, GS)), MULT)
                nc.vector.tensor_tensor(
                    tv, sv, g_mean[:, :, None].to_broadcast((1, G, GS)), MULT)
                nc.vector.tensor_sub(t_row, beta_row, t_row)
                nc.vector.tensor_scalar_mul(s_row, s_row, WS)  # 32*s

                # partition-ize s32, t  ([1,512] row -> [128,4])
                for vec_row, dst in ((s_row, s32_part), (t_row, t_partf)):
                    pp = pize_ps.tile([P, CO], F32, tag="pize", name="pp")
                    for o in range(CO):
                        nc.tensor.matmul(pp[:, o:o + 1],
                                         vec_row[0:1, o * P:(o + 1) * P],
                                         ones_11f,
                                         start=(o == 0), stop=(o == CO - 1))
                    nc.vector.tensor_copy(dst, pp[:])
                nc.vector.tensor_copy(t16_part[:], t_partf)

                # effective biases b' = t @ W + b
                beff_rows = prows.tile([1, 3 * C], F32, tag="beff")
                for i, (name, brow) in enumerate(
                        (("wq", bq_row), ("wk", bk_row), ("wv", bv_row))):
                    bps = stats_ps.tile([1, C], F32, tag="S", name=f"bps{i}")
                    for o in range(CO):
                        nc.tensor.matmul(bps[:], t16_part[:, o:o + 1],
                                         w16[name][:, o, :],
                                         start=(o == 0), stop=(o == CO - 1))
                    nc.vector.tensor_add(beff_rows[:, i * C:(i + 1) * C],
                                         bps[:], brow)
                for i, dst in ((0, bqe_pp), (1, bke_pp)):
                    vec_row = beff_rows[:, i * C:(i + 1) * C]
                    pp = pize_ps.tile([P, CO], F32, tag="pize", name="pp")
                    for o in range(CO):
                        nc.tensor.matmul(pp[:, o:o + 1],
                                         vec_row[0:1, o * P:(o + 1) * P],
                                         ones_11f,
                                         start=(o == 0), stop=(o == CO - 1))
                    nc.vector.tensor_copy(dst[:], pp[:])
                nc.vector.tensor_copy(bv_eff[:], beff_rows[:, 2 * C:3 * C])

                warm(14, "w2a")

                # fold 32*s into fp8 weights
                for i, name in enumerate(("wk", "wq", "wv")):
                    for ci in range(CO):
                        if (i * CO + ci) % 2 == 0:
                            nc.scalar.activation(w8[name][:, ci, :],
                                                 w16[name][:, ci, :], AF.Copy,
                                                 scale=s32_part[:, ci:ci + 1])
                        else:
                            nc.vector.tensor_scalar_mul(
                                w8[name][:, ci, :], w16[name][:, ci, :],
                                s32_part[:, ci:ci + 1])

            # ---- phase 2: Q/K/V projections (DoubleRow fp8) ----
            # K, Q window 0 and V tiles 0..19 are produced here; the
            # remaining V tiles and Q windows migrate into the attention
            # chunks (emitted as `extras`) where the copy engines have slack.
            cpc = [0]

            def copy_sb(dst, srcp, bias_col):
                # gpsimd cannot read PSUM: rotate scalar/vector 4:3
                e = 0 if cpc[0] % 7 < 4 else 1
                cpc[0] += 1
                if bias_col is None:
                    if e == 0:
                        nc.scalar.activation(dst, srcp, AF.Copy,
                                             scale=1.0 / WS)
                    else:
                        nc.vector.tensor_scalar_mul(dst, srcp, 1.0 / WS)
                else:
                    if e == 0:
                        nc.scalar.activation(dst, srcp, AF.Identity,
                                             bias=bias_col, scale=1.0 / WS)
                    else:
                        nc.vector.tensor_scalar(dst, srcp, 1.0 / WS,
                                                bias_col, MULT, ADD)

            def kq_group(pool, ptag, w8t, dstT, bias_pp, o, win, nm):
                ps = pool.tile([P, ICH], F32, tag=ptag,
                               name=f"{nm}{o}_{win}")
                for u in range(2):
                    nc.tensor.matmul(
                        ps[:],
                        w8t[:, 2 * u:2 * u + 2, o * P:(o + 1) * P],
                        xT8[:, 2 * u:2 * u + 2, win * ICH:(win + 1) * ICH],
                        perf_mode=DR, start=(u == 0), stop=(u == 1))
                copy_sb(dstT[:, o, win * ICH:(win + 1) * ICH], ps[:],
                        bias_pp[:, o:o + 1])

            def v_group(pool, ptag, t):
                ps = pool.tile([P, C], F32, tag=ptag, name=f"v{t}")
                for u in range(2):
                    nc.tensor.matmul(
                        ps[:],
                        xT8[:, 2 * u:2 * u + 2, t * P:(t + 1) * P],
                        w8["wv"][:, 2 * u:2 * u + 2, :],
                        perf_mode=DR, start=(u == 0), stop=(u == 1))
                copy_sb(v8[:, t, :], ps[:], None)

            with tc.tile_pool(name="proj_ps", bufs=6, space="PSUM") as proj_ps:
                warm2 = proj_ps.tile([P, P], F32, tag="proj", name="warm2")
                for wi in range(16):
                    nc.tensor.matmul(warm2[:], ones8[:], ones8[:],
                                     perf_mode=DR,
                                     start=(wi == 0), stop=(wi == 15),
                                     skip_group_check=True)
                for o in range(CO):
                    for win in range(N // ICH):
                        kq_group(proj_ps, "proj", w8["wk"], kT8, bke_pp, o, win, "k")
                for o in range(CO):
                    kq_group(proj_ps, "proj", w8["wq"], qT8, bqe_pp, o, 0, "q")
                for t in range(20):
                    v_group(proj_ps, "proj", t)

            # ---- phase 3: attention + O-projection + residual ----
            with (
                tc.tile_pool(name="av_ps", bufs=1, space="PSUM") as av_ps,
                tc.tile_pool(name="sps_ps", bufs=3, space="PSUM") as sps_ps,
                tc.tile_pool(name="op_ps", bufs=1, space="PSUM") as op_ps,
                tc.tile_pool(name="expp", bufs=3) as expp,
                tc.tile_pool(name="accp", bufs=2) as accp,
                tc.tile_pool(name="aoTp", bufs=2) as aoTp,
                tc.tile_pool(name="drow", bufs=2) as drow,
                tc.tile_pool(name="xres", bufs=6) as xres,
                tc.tile_pool(name="ostage", bufs=2) as ostage,
            ):
                LAG = 3  # AV pairs trail scores by 3 so tail MMs interleave

                def make_tail(ch, avs, acc_a, acc_b, last=False):
                    """Chunk-end work, split into pieces emitted between the
                    next chunk's score matmuls (PE queue is in-order, so the
                    tail's dependency waits must be covered by stream MMs)."""
                    st = {}

                    pool, ptag = (sps_ps, "sps") if last else (op_ps, "op")
                    if last:
                        xrs = []
                        for it in range(CO):
                            xr = xres.tile([P, C], F16, tag="xr",
                                           name=f"xrL{it}")
                            nc.sync.dma_start(xr[:], xbo_t[ch * CO + it])
                            xrs.append(xr)

                    def p0():
                        nc.vector.tensor_add(acc_a[:], acc_a[:], acc_b[:])
                        dps = pool.tile([1, ICH], F32, tag=ptag,
                                        name=f"dps{ch}")
                        nc.tensor.matmul(dps[:], ones_col.bitcast(F32),
                                         acc_a[:], start=True, stop=True)
                        d_row = drow.tile([1, ICH], F32R, tag="d_row",
                                          name=f"drow{ch}")
                        nc.vector.tensor_copy(d_row[:], dps[:])
                        st["d_row"] = d_row

                    def p1():
                        d_row = st["d_row"]
                        for cs in range(CO):
                            nc.tensor.matmul(
                                avs[cs][:],
                                bv_eff[0:1, cs * P:(cs + 1) * P], d_row[:],
                                start=False, stop=True)

                    def p2():
                        d_row = st["d_row"]
                        dp = pool.tile([P, CO], F32, tag=ptag, name=f"dp{ch}")
                        for o in range(CO):
                            nc.tensor.matmul(dp[:, o:o + 1],
                                             d_row[0:1, o * P:(o + 1) * P]
                                             .bitcast(F32),
                                             ones_11f,
                                             start=(o == 0),
                                             stop=(o == CO - 1))
                        d_inv = drow.tile([P, CO], F32, tag="d_inv",
                                          name=f"dinv{ch}")
                        nc.vector.tensor_scalar_mul(d_inv[:], dp[:], WS * AOS)
                        nc.vector.reciprocal(d_inv[:], d_inv[:])
                        aoT = aoTp.tile([P, CO, ICH], F8, tag="aoT",
                                        name=f"aoT{ch}")
                        for cs in range(CO):
                            if cs % 2 == 0:
                                nc.vector.tensor_scalar_mul(aoT[:, cs, :],
                                                            avs[cs][:], AOS)
                            else:
                                nc.scalar.activation(aoT[:, cs, :],
                                                     avs[cs][:], AF.Copy,
                                                     scale=AOS)
                        st["d_inv"] = d_inv
                        st["aoT"] = aoT

                    def mk_it(it):
                        def p():
                            aoT, d_inv = st["aoT"], st["d_inv"]
                            ops = pool.tile([P, C], F32, tag=ptag,
                                            name=f"o{ch}_{it}")
                            for u in range(2):
                                nc.tensor.matmul(
                                    ops[:],
                                    aoT[:, 2 * u:2 * u + 2,
                                        it * P:(it + 1) * P],
                                    wo8[:, 2 * u:2 * u + 2, :],
                                    perf_mode=DR, start=(u == 0),
                                    stop=(u == 1))
                            if last:
                                xr = xrs[it]
                            else:
                                xr = xres.tile([P, C], F16, tag="xr",
                                               name=f"xr{ch}_{it}")
                                nc.sync.dma_start(xr[:], xbo_t[ch * CO + it])
                            ot = ostage.tile([P, C], F16, tag="ot",
                                             name=f"ot{ch}_{it}")
                            nc.vector.scalar_tensor_tensor(
                                ot[:], ops[:], d_inv[:, it:it + 1], xr[:],
                                MULT, ADD)
                            nc.sync.dma_start(out_t[ch * CO + it], ot[:])
                        return p

                    return [p0, p1, p2, mk_it(0), mk_it(1), mk_it(2),
                            mk_it(3)]

                extras_by_ch = {
                    0: ([("v", t) for t in range(20, JT)]
                        + [("q", o, 1) for o in range(CO)]),
                    1: [("q", o, 2) for o in range(CO)],
                    2: [("q", o, 3) for o in range(CO)],
                    3: [],
                }

                tail = []
                for ch in range(NCH):
                    i0 = ch * ICH
                    extras = list(extras_by_ch[ch])
                    avs = [av_ps.tile([P, ICH], F32, tag=f"av{i}",
                                      name=f"av{ch}_{i}")
                           for i in range(CO)]
                    acc_a = accp.tile([P, ICH], F32, tag="acc_a",
                                      name=f"acca{ch}")
                    acc_b = accp.tile([P, ICH], F32, tag="acc_b",
                                      name=f"accb{ch}")

                    def scores(j, ex, jj, i0=i0, acc_a=acc_a, acc_b=acc_b,
                               ch=ch):
                        sps = sps_ps.tile([P, ICH], F32, tag="sps",
                                          name=f"sps{ch}_{j}")
                        for u in range(2):
                            nc.tensor.matmul(
                                sps[:],
                                kT8[:, 2 * u:2 * u + 2, j * P:(j + 1) * P],
                                qT8[:, 2 * u:2 * u + 2, i0:i0 + ICH],
                                perf_mode=DR, start=(u == 0), stop=(u == 1))
                        nc.scalar.activation(ex[:, jj, :], sps[:], AF.Exp,
                                             bias=shift_col, scale=SM)
                        if jj == 0:
                            if j == 0:
                                nc.vector.tensor_copy(acc_a[:], ex[:, 0, :])
                            else:
                                nc.vector.tensor_add(acc_a[:], acc_a[:],
                                                     ex[:, 0, :])
                        else:
                            if j == 1:
                                nc.gpsimd.tensor_copy(acc_b[:], ex[:, 1, :])
                            else:
                                nc.gpsimd.tensor_add(acc_b[:], acc_b[:],
                                                     ex[:, 1, :])

                    def av_mms(t, ex, avs=avs):
                        for cs in range(CO):
                            nc.tensor.matmul(
                                avs[cs][:],
                                v8[:, 2 * t:2 * t + 2, cs * P:(cs + 1) * P],
                                ex[:],
                                perf_mode=DR, start=(t == 0), stop=False)

                    lag = 1 if ch == NCH - 1 else LAG
                    exs = {}
                    for t in range(JT // 2):
                        ex = expp.tile([P, 2, ICH], F8, tag="ex",
                                       name=f"ex{ch}_{t}")
                        exs[t] = ex
                        scores(2 * t, ex, 0)
                        scores(2 * t + 1, ex, 1)
                        if 1 <= t <= len(tail):
                            tail[t - 1]()
                        if extras:
                            e = extras.pop(0)
                            if e[0] == "v":
                                v_group(sps_ps, "sps", e[1])
                            else:
                                kq_group(sps_ps, "sps", w8["wq"], qT8,
                                         bqe_pp, e[1], e[2], "qx")
                        if t >= lag:
                            av_mms(t - lag, exs.pop(t - lag))
                    for t in range(JT // 2 - lag, JT // 2):
                        av_mms(t, exs.pop(t))
                    tail = make_tail(ch, avs, acc_a, acc_b,
                                     last=(ch == NCH - 1))
                def warm_tail(n, tag):
                    w = sps_ps.tile([P, P], F32, tag="sps", name=tag)
                    for wi in range(n):
                        nc.tensor.matmul(w[:], ones8[:], ones8[:],
                                         perf_mode=DR,
                                         start=(wi == 0), stop=(wi == n - 1),
                                         skip_group_check=True)

                for i, piece in enumerate(tail):
                    piece()
                    if i in (0, 2):
                        warm_tail(8, f"wt{i}")

    nc.compile()
    return nc


_NC = None


def _get_nc():
    global _NC
    if _NC is None:
        _NC = build_nc()
    return _NC


def make_in_maps(x, gn_gamma, gn_beta, wq, bq, wk, bk, wv, bv, wo, bo):
    x4 = np.asarray(x, np.float32).reshape(B, N, C)

    def wlay(w):
        return np.asarray(w, np.float32).reshape(CO, P, C).transpose(1, 0, 2)

    rows = np.zeros((1, 5 * C), np.float32)
    for i, v in enumerate((gn_gamma, gn_beta, bq, bk, bv)):
        rows[0, i * C:(i + 1) * C] = np.asarray(v, np.float32)
    cst = np.zeros((P, 3), np.float32)
    cst[:, 0] = 1.0
    cst[:, 1] = -SHIFT
    cst[:, 2] = EPS
    # col 2: group-norm eps (bias operand of the Sqrt activation)
    common = dict(
        wq16=np.ascontiguousarray(wlay(wq).astype(np.float16)),
        wk16=np.ascontiguousarray(wlay(wk).astype(np.float16)),
        wv16=np.ascontiguousarray(wlay(wv).astype(np.float16)),
        wo8=np.ascontiguousarray((WS * wlay(wo)).astype(F8NP)),
        rows=rows, cst=cst,
    )
    bo_f = np.asarray(bo, np.float32)
    in_maps = []
    for c in range(N_CORES):
        b, h = c // 2, c % 2
        own = x4[b, h * HALF:(h + 1) * HALF]
        other = x4[b, (1 - h) * HALF:(2 - h) * HALF]
        xp = np.concatenate([own, other], axis=0)        # [N, C]
        xT8 = np.ascontiguousarray(xp.T.astype(F8NP))    # [C, N]
        xi = xp.reshape(RT, 2, P, C).transpose(0, 2, 1, 3) \
               .reshape(RT * P, 2 * C)
        x8i = np.ascontiguousarray(xi.astype(F8NP))
        xbo = np.ascontiguousarray((own + bo_f).astype(np.float16))
        in_maps.append(dict(xT8=xT8, x8i=x8i, xbo=xbo, **common))
    return in_maps


def assemble(results):
    out = np.empty((B, N, C), np.float32)
    for c in range(N_CORES):
        b, h = c // 2, c % 2
        out[b, h * HALF:(h + 1) * HALF] = results[c]["out"].astype(np.float32)
    return out.reshape(B, 64, 64, C)


def kernel(**inputs):
    nc = _get_nc()
    in_maps = make_in_maps(**inputs)
    res = run_bass_kernel_spmd(nc, in_maps, list(range(N_CORES)))
    return assemble(res.results)


# revision 31
# speedup vs baseline: 1.1975x; 1.0076x over previous
"""Trainium2 Bass kernel for a spatial self-attention block (fp8 DoubleRow).

reference computation (B=4, H=W=64, C=512, N=H*W=4096):
    h = group_norm(x, gamma, beta, 32 groups)
    q,k,v = h@wq+bq, h@wk+bk, h@wv+bv
    scores = (q @ k^T) / sqrt(C); attn = softmax(scores, -1)
    out = (attn @ v) @ wo + bo + x

Sharding: 8 cores = (batch b in 0..3) x (query-half in 0..1). Each core
computes group-norm stats + K/V for its full batch element (duplicated
across the pair) and attention outputs for its own 2048 query rows. The
host permutes each core's batch rows so its own queries are rows 0:2048.

All heavy matmuls run in fp8(e4m3) with perf_mode=DoubleRow: operands are
3D APs [128, 2, free] and the PE contracts over (partition x pair), giving
2 MACs/cell/cycle (~1.8x fp16 matmul throughput at free-dim 512).

Precision scheme (validated vs the fp32 reference: rel err ~9e-3 against a
2e-2 budget):
  - x arrives pre-transposed and pre-pair-interleaved from the host in fp8.
  - group-norm stats come from fp8 x and fp8 squares via DoubleRow matmuls
    against an all-ones stationary; scale/shift s,t are fp32 on-device.
  - s is folded into fp8 copies of wq/wk/wv scaled by WS=32 (weight entries
    ~N(0, 1/C) are too small for e4m3 otherwise); the 1/WS is applied in
    the PSUM->SBUF copy.  t is folded into effective biases (t@w + b).
  - exp uses a fixed shift: ex = exp(s/sqrt(C) - SHIFT), stored fp8
    (max scaled score measured ~6.8 -> e^4.8 = 127 < 240 = e4m3 max).
    The shift cancels in softmax normalization.
  - attn@V is computed unnormalized; V bias enters as bv_eff (x) denom
    (rows of unnormalized softmax sum to denom); the result is scaled by
    AOS=1/64 into fp8 for the O-projection, and 1/(WS*AOS*denom) is
    applied per-query after the O-projection.
"""

import sys

import numpy as np
import ml_dtypes

if "/opt/trn_rl_repo" not in sys.path:
    sys.path.insert(0, "/opt/trn_rl_repo")

import concourse.mybir as mybir
import concourse.tile as tile
from concourse import bacc
from concourse.bass_utils import run_bass_kernel_spmd

F32 = mybir.dt.float32
F32R = mybir.dt.float32r
F16 = mybir.dt.float16
F8 = mybir.dt.float8e4
AF = mybir.ActivationFunctionType
DR = mybir.MatmulPerfMode.DoubleRow
MULT = mybir.AluOpType.mult
ADD = mybir.AluOpType.add

B, N, C = 4, 4096, 512
HALF = N // 2          # own query rows per core
G, GS = 32, 16         # groups, channels per group
P = 128                # partitions
CO = C // P            # channel subtiles (4)
N_CORES = 8
EPS = 1e-6
SM = 1.0 / float(np.sqrt(C))
WS = 32.0              # weight fp8 scale
SHIFT = 2.0            # exp shift (cancels in softmax)
AOS = 1.0 / 64.0       # attn-output fp8 scale
ICH = 512              # query chunk
NCH = HALF // ICH      # 4
JT = N // P            # 32 key tiles
RT = N // 256          # 16 row-pair tiles (stats)
F8NP = ml_dtypes.float8_e4m3


def _r(ap):
    return ap.bitcast(F32R)


def build_nc():
    nc = bacc.Bacc("TRN2", target_bir_lowering=False, num_devices=N_CORES)

    xT8_d = nc.dram_tensor("xT8", [C, N], F8, kind="ExternalInput")
    x8i_d = nc.dram_tensor("x8i", [RT * P, 2 * C], F8, kind="ExternalInput")
    wq16_d = nc.dram_tensor("wq16", [P, CO, C], F16, kind="ExternalInput")
    wk16_d = nc.dram_tensor("wk16", [P, CO, C], F16, kind="ExternalInput")
    wv16_d = nc.dram_tensor("wv16", [P, CO, C], F16, kind="ExternalInput")
    wo8_d = nc.dram_tensor("wo8", [P, CO, C], F8, kind="ExternalInput")
    rows_d = nc.dram_tensor("rows", [1, 5 * C], F32, kind="ExternalInput")
    cst_d = nc.dram_tensor("cst", [P, 3], F32R, kind="ExternalInput")
    xbo_d = nc.dram_tensor("xbo", [HALF, C], F16, kind="ExternalInput")
    out_d = nc.dram_tensor("out", [HALF, C], F16, kind="ExternalOutput")

    x8i_b = x8i_d[:].rearrange("(b t p) c -> b p t c", t=4, p=P)  # 4x[128,4,1024]
    xbo_t = xbo_d[:].rearrange("(t p) c -> t p c", p=P)   # 16 x [128, 512]
    out_t = out_d[:].rearrange("(t p) c -> t p c", p=P)   # 16 x [128, 512]

    with tile.TileContext(nc) as tc:
        with (
            tc.tile_pool(name="persist", bufs=1) as persist,
            tc.tile_pool(name="cpool", bufs=1) as cpool,
        ):
            xT8 = persist.tile([P, CO, N], F8, tag="xT8")
            kT8 = persist.tile([P, CO, N], F8, tag="kT8")
            qT8 = persist.tile([P, CO, HALF], F8, tag="qT8")
            v8 = persist.tile([P, JT, C], F8, tag="v8")

            cst = cpool.tile([P, 3], F32R, tag="cst")
            ones8 = cpool.tile([P, 2, P], F8, tag="ones8")
            wo8 = cpool.tile([P, CO, C], F8, tag="wo8")
            w8 = {n: cpool.tile([P, CO, C], F8, tag=f"w8{n}", name=f"w8{n}")
                  for n in ("wq", "wk", "wv")}
            bqe_pp = cpool.tile([P, CO], F32, tag="bqe")
            bke_pp = cpool.tile([P, CO], F32, tag="bke")
            bv_eff = cpool.tile([1, C], F32R, tag="bve")

            nc.sync.dma_start(cst[:], cst_d[:])
            nc.gpsimd.memset(ones8[:], 1.0)
            ones_col = cst[:, 0:1]            # F32R
            ones_11 = cst[0:1, 0:1]           # F32R
            ones_11f = cst[0:1, 0:1].bitcast(F32)
            shift_col = cst[:, 1:2].bitcast(F32)
            eps_col = cst[:, 2:3].bitcast(F32)

            # ---- phase 1: group-norm stats + weight folding ----
            with (
                tc.tile_pool(name="w16p", bufs=1) as w16p,
                tc.tile_pool(name="xstage", bufs=4) as xstage,
                tc.tile_pool(name="sqpool", bufs=4) as sqpool,
                tc.tile_pool(name="prows", bufs=1) as prows,
                tc.tile_pool(name="stats_ps", bufs=1, space="PSUM") as stats_ps,
                tc.tile_pool(name="pize_ps", bufs=1, space="PSUM") as pize_ps,
                tc.tile_pool(name="warm_ps", bufs=1, space="PSUM") as warm_ps,
            ):
                # keep the PE busy from t~0 so the HAM clock gate opens
                # (K=8/8) before the real matmuls arrive; result never read.
                def warm(n, tag):
                    w = warm_ps.tile([P, P], F32, tag="warm", name=tag)
                    for wi in range(n):
                        nc.tensor.matmul(w[:], ones8[:], ones8[:],
                                         perf_mode=DR,
                                         start=(wi == 0), stop=(wi == n - 1),
                                         skip_group_check=True)

                warm(24, "w0")
                w16 = {n: w16p.tile([P, CO, C], F16, tag=f"w16{n}",
                                    name=f"w16{n}")
                       for n in ("wq", "wk", "wv")}

                irows = prows.tile([1, 5 * C], F32, tag="irows")
                nc.sync.dma_start(irows[:], rows_d[:])
                gamma_row = irows[:, 0 * C:1 * C]
                beta_row = irows[:, 1 * C:2 * C]
                bq_row = irows[:, 2 * C:3 * C]
                bk_row = irows[:, 3 * C:4 * C]
                bv_row = irows[:, 4 * C:5 * C]
                wrows = prows.tile([1, 4 * C], F32, tag="wrows")
                sum_row = wrows[:, 0 * C:1 * C]
                sq_row = wrows[:, 1 * C:2 * C]
                s_row = wrows[:, 2 * C:3 * C]
                t_row = wrows[:, 3 * C:4 * C]
                grows = prows.tile([1, 3 * G], F32, tag="grows")
                g_mean = grows[:, 0:G]
                g_var = grows[:, G:2 * G]
                g_tmp = grows[:, 2 * G:3 * G]
                stpart = prows.tile([P, 2 * CO], F32, tag="stpart")
                s32_part = stpart[:, 0:CO]
                t_partf = stpart[:, CO:2 * CO]
                t16_part = prows.tile([P, CO], F16, tag="t16")

                # stats: column sums and sums-of-squares via DoubleRow
                s_ps = stats_ps.tile([P, C], F32, tag="S")
                q_ps = stats_ps.tile([P, C], F32, tag="Q")
                NB = 4  # row-pair tiles per DMA batch
                sq_eng = [nc.vector, nc.gpsimd, nc.scalar, nc.scalar,
                          nc.vector, nc.gpsimd, nc.scalar, nc.scalar,
                          nc.vector, nc.gpsimd, nc.scalar, nc.scalar,
                          nc.scalar, nc.scalar, nc.scalar, nc.scalar]
                sqs = []
                for b in range(RT // NB):
                    xt = xstage.tile([P, NB, 2, C], F8, tag="xt",
                                     name=f"xt{b}")
                    sq = sqpool.tile([P, NB, 2, C], F8, tag="sq",
                                     name=f"sq{b}")
                    sqs.append(sq)
                    if b % 2 == 0:
                        nc.sync.dma_start(
                            xt[:].rearrange("p t two c -> p t (two c)"),
                            x8i_b[b])
                    else:
                        nc.gpsimd.dma_start(
                            xt[:].rearrange("p t two c -> p t (two c)"),
                            x8i_b[b])
                    for t in range(NB):
                        g = b * NB + t
                        nc.tensor.matmul(s_ps[:], ones8[:], xt[:, t],
                                         perf_mode=DR,
                                         start=(g == 0), stop=(g == RT - 1))
                        e = sq_eng[g]
                        if e is nc.scalar:
                            e.activation(sq[:, t], xt[:, t], AF.Square)
                        else:
                            e.tensor_mul(sq[:, t], xt[:, t], xt[:, t])
                    warm(6, f"wb{b}")
                # squares complete on scalar/vector/gpsimd while the sum
                # matmuls stream; the sumsq matmuls then run back-to-back
                for g in range(RT):
                    nc.tensor.matmul(q_ps[:], ones8[:], sqs[g // NB][:, g % NB],
                                     perf_mode=DR,
                                     start=(g == 0), stop=(g == RT - 1))
                for o in range(CO):
                    nc.sync.dma_start(xT8[:, o, :],
                                      xT8_d[o * P:(o + 1) * P, :])
                for name, srcd in (("wk", wk16_d), ("wq", wq16_d),
                                   ("wv", wv16_d)):
                    nc.sync.dma_start(w16[name][:], srcd[:])
                nc.sync.dma_start(wo8[:], wo8_d[:])

                # warm filler: PE stays hot while the rows chain runs
                warm(24, "w1")

                # group stats -> per-channel scale/shift (rows, DVE)
                inv_cnt = 1.0 / (N * GS)
                nc.vector.reduce_sum(g_mean,
                                     s_ps[0:1, :].rearrange(
                                         "p (g e) -> p g e", e=GS),
                                     axis=mybir.AxisListType.X)
                nc.vector.tensor_scalar_mul(g_mean, g_mean, inv_cnt)
                nc.vector.reduce_sum(g_var,
                                     q_ps[0:1, :].rearrange(
                                         "p (g e) -> p g e", e=GS),
                                     axis=mybir.AxisListType.X)
                nc.vector.tensor_mul(g_tmp, g_mean, g_mean)
                nc.vector.scalar_tensor_tensor(g_var, g_var, inv_cnt, g_tmp,
                                               MULT, mybir.AluOpType.subtract)
                nc.scalar.activation(g_tmp, g_var, AF.Sqrt,
                                     bias=eps_col[0:1, :])
                nc.vector.reciprocal(g_tmp, g_tmp)  # rstd per group

                sv = s_row.rearrange("p (g e) -> p g e", e=GS)
                tv = t_row.rearrange("p (g e) -> p g e", e=GS)
                gv = gamma_row.rearrange("p (g e) -> p g e", e=GS)
                nc.vector.tensor_tensor(
                    sv, gv, g_tmp[:, :, None].to_broadcast((1, G, GS)), MULT)
                nc.vector.tensor_tensor(
                    tv, sv, g_mean[:, :, None].to_broadcast((1, G, GS)), MULT)
                nc.vector.tensor_sub(t_row, beta_row, t_row)
                nc.vector.tensor_scalar_mul(s_row, s_row, WS)  # 32*s

                # partition-ize s32, t  ([1,512] row -> [128,4])
                for vec_row, dst in ((s_row, s32_part), (t_row, t_partf)):
                    pp = pize_ps.tile([P, CO], F32, tag="pize", name="pp")
                    for o in range(CO):
                        nc.tensor.matmul(pp[:, o:o + 1],
                                         vec_row[0:1, o * P:(o + 1) * P],
                                         ones_11f,
                                         start=(o == 0), stop=(o == CO - 1))
                    nc.vector.tensor_copy(dst, pp[:])
                nc.vector.tensor_copy(t16_part[:], t_partf)

                # effective biases b' = t @ W + b
                beff_rows = prows.tile([1, 3 * C], F32, tag="beff")
                for i, (name, brow) in enumerate(
                        (("wq", bq_row), ("wk", bk_row), ("wv", bv_row))):
                    bps = stats_ps.tile([1, C], F32, tag="S", name=f"bps{i}")
                    for o in range(CO):
                        nc.tensor.matmul(bps[:], t16_part[:, o:o + 1],
                                         w16[name][:, o, :],
                                         start=(o == 0), stop=(o == CO - 1))
                    nc.vector.tensor_add(beff_rows[:, i * C:(i + 1) * C],
                                         bps[:], brow)
                for i, dst in ((0, bqe_pp), (1, bke_pp)):
                    vec_row = beff_rows[:, i * C:(i + 1) * C]
                    pp = pize_ps.tile([P, CO], F32, tag="pize", name="pp")
                    for o in range(CO):
                        nc.tensor.matmul(pp[:, o:o + 1],
                                         vec_row[0:1, o * P:(o + 1) * P],
                                         ones_11f,
                                         start=(o == 0), stop=(o == CO - 1))
                    nc.vector.tensor_copy(dst[:], pp[:])
                nc.vector.tensor_copy(bv_eff[:], beff_rows[:, 2 * C:3 * C])

                warm(14, "w2a")

                # fold 32*s into fp8 weights
                for i, name in enumerate(("wk", "wq", "wv")):
                    for ci in range(CO):
                        if (i * CO + ci) % 2 == 0:
                            nc.scalar.activation(w8[name][:, ci, :],
                                                 w16[name][:, ci, :], AF.Copy,
                                                 scale=s32_part[:, ci:ci + 1])
                        else:
                            nc.vector.tensor_scalar_mul(
                                w8[name][:, ci, :], w16[name][:, ci, :],
                                s32_part[:, ci:ci + 1])

            # ---- phase 2: Q/K/V projections (DoubleRow fp8) ----
            # K, Q window 0 and V tiles 0..19 are produced here; the
            # remaining V tiles and Q windows migrate into the attention
            # chunks (emitted as `extras`) where the copy engines have slack.
            cpc = [0]

            def copy_sb(dst, srcp, bias_col):
                # gpsimd cannot read PSUM: rotate scalar/vector 4:3
                e = 0 if cpc[0] % 7 < 4 else 1
                cpc[0] += 1
                if bias_col is None:
                    if e == 0:
                        nc.scalar.activation(dst, srcp, AF.Copy,
                                             scale=1.0 / WS)
                    else:
                        nc.vector.tensor_scalar_mul(dst, srcp, 1.0 / WS)
                else:
                    if e == 0:
                        nc.scalar.activation(dst, srcp, AF.Identity,
                                             bias=bias_col, scale=1.0 / WS)
                    else:
                        nc.vector.tensor_scalar(dst, srcp, 1.0 / WS,
                                                bias_col, MULT, ADD)

            def kq_group(pool, ptag, w8t, dstT, bias_pp, o, win, nm):
                ps = pool.tile([P, ICH], F32, tag=ptag,
                               name=f"{nm}{o}_{win}")
                for u in range(2):
                    nc.tensor.matmul(
                        ps[:],
                        w8t[:, 2 * u:2 * u + 2, o * P:(o + 1) * P],
                        xT8[:, 2 * u:2 * u + 2, win * ICH:(win + 1) * ICH],
                        perf_mode=DR, start=(u == 0), stop=(u == 1))
                copy_sb(dstT[:, o, win * ICH:(win + 1) * ICH], ps[:],
                        bias_pp[:, o:o + 1])

            def v_group(pool, ptag, t):
                ps = pool.tile([P, C], F32, tag=ptag, name=f"v{t}")
                for u in range(2):
                    nc.tensor.matmul(
                        ps[:],
                        xT8[:, 2 * u:2 * u + 2, t * P:(t + 1) * P],
                        w8["wv"][:, 2 * u:2 * u + 2, :],
                        perf_mode=DR, start=(u == 0), stop=(u == 1))
                copy_sb(v8[:, t, :], ps[:], None)

            with tc.tile_pool(name="proj_ps", bufs=6, space="PSUM") as proj_ps:
                warm2 = proj_ps.tile([P, P], F32, tag="proj", name="warm2")
                for wi in range(16):
                    nc.tensor.matmul(warm2[:], ones8[:], ones8[:],
                                     perf_mode=DR,
                                     start=(wi == 0), stop=(wi == 15),
                                     skip_group_check=True)
                for o in range(CO):
                    for win in range(N // ICH):
                        kq_group(proj_ps, "proj", w8["wk"], kT8, bke_pp, o, win, "k")
                for o in range(CO):
                    kq_group(proj_ps, "proj", w8["wq"], qT8, bqe_pp, o, 0, "q")
                for t in range(20):
                    v_group(proj_ps, "proj", t)

            # ---- phase 3: attention + O-projection + residual ----
            with (
                tc.tile_pool(name="av_ps", bufs=1, space="PSUM") as av_ps,
                tc.tile_pool(name="sps_ps", bufs=3, space="PSUM") as sps_ps,
                tc.tile_pool(name="op_ps", bufs=1, space="PSUM") as op_ps,
                tc.tile_pool(name="expp", bufs=3) as expp,
                tc.tile_pool(name="accp", bufs=2) as accp,
                tc.tile_pool(name="aoTp", bufs=2) as aoTp,
                tc.tile_pool(name="drow", bufs=2) as drow,
                tc.tile_pool(name="xres", bufs=6) as xres,
                tc.tile_pool(name="ostage", bufs=2) as ostage,
            ):
                LAG = 3  # AV pairs trail scores by 3 so tail MMs interleave

                def make_tail(ch, avs, acc_a, acc_b, last=False):
                    """Chunk-end work, split into pieces emitted between the
                    next chunk's score matmuls (PE queue is in-order, so the
                    tail's dependency waits must be covered by stream MMs)."""
                    st = {}

                    pool, ptag = (sps_ps, "sps") if last else (op_ps, "op")
                    if last:
                        xrs = []
                        for it in range(CO):
                            xr = xres.tile([P, C], F16, tag="xr",
                                           name=f"xrL{it}")
                            nc.sync.dma_start(xr[:], xbo_t[ch * CO + it])
                            xrs.append(xr)

                    def p0():
                        nc.vector.tensor_add(acc_a[:], acc_a[:], acc_b[:])
                        dps = pool.tile([1, ICH], F32, tag=ptag,
                                        name=f"dps{ch}")
                        nc.tensor.matmul(dps[:], ones_col.bitcast(F32),
                                         acc_a[:], start=True, stop=True)
                        d_row = drow.tile([1, ICH], F32R, tag="d_row",
                                          name=f"drow{ch}")
                        nc.vector.tensor_copy(d_row[:], dps[:])
                        st["d_row"] = d_row

                    def p1():
                        d_row = st["d_row"]
                        for cs in range(CO):
                            nc.tensor.matmul(
                                avs[cs][:],
                                bv_eff[0:1, cs * P:(cs + 1) * P], d_row[:],
                                start=False, stop=True)

                    def p2():
                        d_row = st["d_row"]
                        dp = pool.tile([P, CO], F32, tag=ptag, name=f"dp{ch}")
                        for o in range(CO):
                            nc.tensor.matmul(dp[:, o:o + 1],
                                             d_row[0:1, o * P:(o + 1) * P]
                                             .bitcast(F32),
                                             ones_11f,
                                             start=(o == 0),
                                             stop=(o == CO - 1))
                        d_inv = drow.tile([P, CO], F32, tag="d_inv",
                                          name=f"dinv{ch}")
                        nc.vector.tensor_scalar_mul(d_inv[:], dp[:], WS * AOS)
                        nc.vector.reciprocal(d_inv[:], d_inv[:])
                        aoT = aoTp.tile([P, CO, ICH], F8, tag="aoT",
                                        name=f"aoT{ch}")
                        for cs in range(CO):
                            if cs % 2 == 0:
                                nc.vector.tensor_scalar_mul(aoT[:, cs, :],
                                                            avs[cs][:], AOS)
                            else:
                                nc.scalar.activation(aoT[:, cs, :],
                                                     avs[cs][:], AF.Copy,
                                                     scale=AOS)
                        st["d_inv"] = d_inv
                        st["aoT"] = aoT

                    def mk_it(it):
                        def p():
                            aoT, d_inv = st["aoT"], st["d_inv"]
                            ops = pool.tile([P, C], F32, tag=ptag,
                                            name=f"o{ch}_{it}")
                            for u in range(2):
                                nc.tensor.matmul(
                                    ops[:],
                                    aoT[:, 2 * u:2 * u + 2,
                                        it * P:(it + 1) * P],
                                    wo8[:, 2 * u:2 * u + 2, :],
                                    perf_mode=DR, start=(u == 0),
                                    stop=(u == 1))
                            if last:
                                xr = xrs[it]
                            else:
                                xr = xres.tile([P, C], F16, tag="xr",
                                               name=f"xr{ch}_{it}")
                                nc.sync.dma_start(xr[:], xbo_t[ch * CO + it])
                            ot = ostage.tile([P, C], F16, tag="ot",
                                             name=f"ot{ch}_{it}")
                            nc.vector.scalar_tensor_tensor(
                                ot[:], ops[:], d_inv[:, it:it + 1], xr[:],
                                MULT, ADD)
                            nc.sync.dma_start(out_t[ch * CO + it], ot[:])
                        return p

                    return [p0, p1, p2, mk_it(0), mk_it(1), mk_it(2),
                            mk_it(3)]

                extras_by_ch = {
                    0: ([("v", t) for t in range(20, JT)]
                        + [("q", o, 1) for o in range(CO)]),
                    1: [("q", o, 2) for o in range(CO)],
                    2: [("q", o, 3) for o in range(CO)],
                    3: [],
                }

                tail = []
                for ch in range(NCH):
                    i0 = ch * ICH
                    extras = list(extras_by_ch[ch])
                    avs = [av_ps.tile([P, ICH], F32, tag=f"av{i}",
                                      name=f"av{ch}_{i}")
                           for i in range(CO)]
                    acc_a = accp.tile([P, ICH], F32, tag="acc_a",
                                      name=f"acca{ch}")
                    acc_b = accp.tile([P, ICH], F32, tag="acc_b",
                                      name=f"accb{ch}")

                    def scores(j, ex, jj, i0=i0, acc_a=acc_a, acc_b=acc_b,
                               ch=ch):
                        sps = sps_ps.tile([P, ICH], F32, tag="sps",
                                          name=f"sps{ch}_{j}")
                        for u in range(2):
                            nc.tensor.matmul(
                                sps[:],
                                kT8[:, 2 * u:2 * u + 2, j * P:(j + 1) * P],
                                qT8[:, 2 * u:2 * u + 2, i0:i0 + ICH],
                                perf_mode=DR, start=(u == 0), stop=(u == 1))
                        nc.scalar.activation(ex[:, jj, :], sps[:], AF.Exp,
                                             bias=shift_col, scale=SM)
                        if jj == 0:
                            if j == 0:
                                nc.vector.tensor_copy(acc_a[:], ex[:, 0, :])
                            else:
                                nc.vector.tensor_add(acc_a[:], acc_a[:],
                                                     ex[:, 0, :])
                        else:
                            if j == 1:
                                nc.gpsimd.tensor_copy(acc_b[:], ex[:, 1, :])
                            else:
                                nc.gpsimd.tensor_add(acc_b[:], acc_b[:],
                                                     ex[:, 1, :])

                    def av_mms(t, ex, avs=avs):
                        for cs in range(CO):
                            nc.tensor.matmul(
                                avs[cs][:],
                                v8[:, 2 * t:2 * t + 2, cs * P:(cs + 1) * P],
                                ex[:],
                                perf_mode=DR, start=(t == 0), stop=False)

                    lag = 1 if ch == NCH - 1 else LAG
                    exs = {}
                    for t in range(JT // 2):
                        ex = expp.tile([P, 2, ICH], F8, tag="ex",
                                       name=f"ex{ch}_{t}")
                        exs[t] = ex
                        scores(2 * t, ex, 0)
                        scores(2 * t + 1, ex, 1)
                        if 1 <= t <= len(tail):
                            tail[t - 1]()
                        if extras:
                            e = extras.pop(0)
                            if e[0] == "v":
                                v_group(sps_ps, "sps", e[1])
                            else:
                                kq_group(sps_ps, "sps", w8["wq"], qT8,
                                         bqe_pp, e[1], e[2], "qx")
                        if t >= lag:
                            av_mms(t - lag, exs.pop(t - lag))
                    for t in range(JT // 2 - lag, JT // 2):
                        av_mms(t, exs.pop(t))
                    tail = make_tail(ch, avs, acc_a, acc_b,
                                     last=(ch == NCH - 1))
                def warm_tail(n, tag):
                    w = sps_ps.tile([P, P], F32, tag="sps", name=tag)
                    for wi in range(n):
                        nc.tensor.matmul(w[:], ones8[:], ones8[:],
                                         perf_mode=DR,
                                         start=(wi == 0), stop=(wi == n - 1),
                                         skip_group_check=True)

                for i, piece in enumerate(tail):
                    piece()
                    if i in (0, 2):
                        warm_tail(8, f"wt{i}")

    nc.compile()
    return nc


_NC = None


def _get_nc():
    global _NC
    if _NC is None:
        _NC = build_nc()
    return _NC


def make_in_maps(x, gn_gamma, gn_beta, wq, bq, wk, bk, wv, bv, wo, bo):
    x4 = np.asarray(x, np.float32).reshape(B, N, C)

    def wlay(w):
        return np.asarray(w, np.float32).reshape(CO, P, C).transpose(1, 0, 2)

    rows = np.zeros((1, 5 * C), np.float32)
    for i, v in enumerate((gn_gamma, gn_beta, bq, bk, bv)):
        rows[0, i * C:(i + 1) * C] = np.asarray(v, np.float32)
    cst = np.zeros((P, 3), np.float32)
    cst[:, 0] = 1.0
    cst[:, 1] = -SHIFT
    cst[:, 2] = EPS
    # col 2: group-norm eps (bias operand of the Sqrt activation)
    common = dict(
        wq16=np.ascontiguousarray(wlay(wq).astype(np.float16)),
        wk16=np.ascontiguousarray(wlay(wk).astype(np.float16)),
        wv16=np.ascontiguousarray(wlay(wv).astype(np.float16)),
        wo8=np.ascontiguousarray((WS * wlay(wo)).astype(F8NP)),
        rows=rows, cst=cst,
    )
    bo_f = np.asarray(bo, np.float32)
    in_maps = []
    for c in range(N_CORES):
        b, h = c // 2, c % 2
        own = x4[b, h * HALF:(h + 1) * HALF]
        other = x4[b, (1 - h) * HALF:(2 - h) * HALF]
        xp = np.concatenate([own, other], axis=0)        # [N, C]
        xT8 = np.ascontiguousarray(xp.T.astype(F8NP))    # [C, N]
        xi = xp.reshape(RT, 2, P, C).transpose(0, 2, 1, 3) \
               .reshape(RT * P, 2 * C)
        x8i = np.ascontiguousarray(xi.astype(F8NP))
        xbo = np.ascontiguousarray((own + bo_f).astype(np.float16))
        in_maps.append(dict(xT8=xT8, x8i=x8i, xbo=xbo, **common))
    return in_maps


def assemble(results):
    out = np.empty((B, N, C), np.float32)
    for c in range(N_CORES):
        b, h = c // 2, c % 2
        out[b, h * HALF:(h + 1) * HALF] = results[c]["out"].astype(np.float32)
    return out.reshape(B, 64, 64, C)


def kernel(**inputs):
    nc = _get_nc()
    in_maps = make_in_maps(**inputs)
    res = run_bass_kernel_spmd(nc, in_maps, list(range(N_CORES)))
    return assemble(res.results)
